# revision 24
# baseline (speedup 1.0000x reference)
"""MoE layer (top-2 of 8 experts, d_model=1024, d_ff=4096) on 8 TRN2 NeuronCores.

Strategy: expert parallelism. The gate (x @ Wg + bg, top-2, softmax) is the
sharding decision and runs on host in fp32 (bit-stable vs the reference's
fp32 gate: min |logit2 - logit3| gap for these inputs is ~7e-5, far above
fp32 matmul rounding). Each core e receives the tokens routed to expert e
(padded to a common capacity C = max expert load, 8-aligned), W1/b1/W2/b2 of
expert e, and the per-token combine weight. The device computes

    yT = (relu(W1^T @ xT + b1)^T @ W2 + b2)^T * cw      (bf16 mm, fp32 acc)

entirely in transposed (feature-major) layout so both biases land on the
partition axis (scalar-engine activation bias) and the combine weight lands
on the free axis (one fused DVE op). Host scatter-adds the two expert
contributions per token (indices within one expert are unique, so fancy
indexing += is exact).
"""

import os
import sys

import numpy as np
import ml_dtypes

B, S, D, F, E, TOPK = 4, 2048, 1024, 4096, 8, 2
P = 128
KD = D // P  # 8  k-tiles over d_model
KF = F // P  # 32 k-tiles over d_ff

_PROG_CACHE: dict = {}
LAST_RESULTS = None  # BassKernelResults of the most recent device run


def _chunk_sizes(C):
    # Chunks of <=512 tokens (PSUM bank limit / max fp32 matmul free dim).
    # Weights are SBUF-resident, so a small tail chunk costs only its own
    # per-matmul overhead.
    sizes = []
    c = C
    while c > 0:
        n = min(512, c)
        sizes.append(n)
        c -= n
    return sizes


def _build_program(C, d=D, f=F):
    """One SPMD program: FFN for `C` tokens through one expert (transposed layout).

    DRAM inputs (per core):
      xT  [P, kd, C]    bf16   xT[p,k,c] = x[c, k*128+p]
      w1  [P, kf, kd*P] bf16   w1[p,j,k*128+q] = W1[k*128+p, j*128+q]  (m-major)
      b1  [P, kf]       f32    b1[p,j] = b1[j*128+p]
      w2  [P, kf, d]    bf16   w2[p,k,:] = W2[k*128+p, :]              (k-major)
      b2  [P, kd]       f32    b2[p,j] = b2[j*128+p]
      cw  [P, C]        f32    combine weight per token, replicated over partitions
    Output:
      out [P, kd, C]    f32    out[p,j,c] = y[c, j*128+p]
    """
    from contextlib import ExitStack

    import concourse.tile as tile
    from concourse import bacc, mybir

    kd = d // P
    kf = f // P
    dt = mybir.dt
    AF = mybir.ActivationFunctionType
    Alu = mybir.AluOpType

    nc = bacc.Bacc("TRN2", target_bir_lowering=False, debug=False, num_devices=E)
    xT = nc.dram_tensor("xT", [P, kd, C], dt.bfloat16, kind="ExternalInput").ap()
    w1 = nc.dram_tensor("w1", [P, kf, kd * P], dt.bfloat16, kind="ExternalInput").ap()
    b1 = nc.dram_tensor("b1", [P, kf], dt.float32, kind="ExternalInput").ap()
    w2 = nc.dram_tensor("w2", [P, kf, d], dt.bfloat16, kind="ExternalInput").ap()
    b2 = nc.dram_tensor("b2", [P, kd], dt.float32, kind="ExternalInput").ap()
    cw = nc.dram_tensor("cw", [P, C], dt.float32, kind="ExternalInput").ap()
    out = nc.dram_tensor("out", [P, kd, C], dt.float32, kind="ExternalOutput").ap()

    with tile.TileContext(nc) as tc, ExitStack() as ctx:
        xp = ctx.enter_context(tc.tile_pool(name="xp", bufs=2))
        hp = ctx.enter_context(tc.tile_pool(name="hp", bufs=1))
        yp = ctx.enter_context(tc.tile_pool(name="yp", bufs=3))
        cp = ctx.enter_context(tc.tile_pool(name="cp", bufs=1))
        pp = ctx.enter_context(tc.tile_pool(name="pp", bufs=8, space="PSUM"))

        # Both weight matrices are SBUF-resident (128KB/partition total).
        # Constants + W2 load on the scalar HWDGE ring, so the sync ring is
        # free for the latency-critical W1/x loads (first MM fires a few us
        # in; W2 is only needed ~70us in, once FFN2 starts). W2 is loaded in
        # 256KB slices — a single 8.4MB HWDGE DMA faults the device.
        b1_sb = cp.tile([P, kf], dt.float32)
        nc.scalar.dma_start(b1_sb[:], b1[:])
        b2_sb = cp.tile([P, kd], dt.float32)
        nc.scalar.dma_start(b2_sb[:], b2[:])
        cw_sb = cp.tile([P, C], dt.float32)
        w1_sb = cp.tile([P, kf, kd * P], dt.bfloat16)
        w2_sb = cp.tile([P, kf, d], dt.bfloat16)
        # W2 loads trail W1 consumption: slices 0-3 up front, then slice j+4
        # after chunk-0 eviction j (issued from the scalar queue, so program
        # order defers them). This keeps the bulk load off the SDMA engines
        # during the W1/x ramp (which starved FFN1 and re-throttled the PE)
        # while staying 4 slices ahead of FFN2's first pass. cw (1.1MB) is
        # likewise deferred to mid-chunk-0, ~30us before its first use.
        for k in range(min(4, kf)):
            nc.scalar.dma_start(w2_sb[:, k : k + 1, :], w2[:, k : k + 1, :])

        c0 = 0
        for ci, n in enumerate(_chunk_sizes(C)):
            # x on the gpsimd (SWDGE) ring: parallel with w1_0 on sync, so
            # the first matmul's two inputs don't serialize on one queue.
            x_sb = xp.tile([P, kd, n], dt.bfloat16, tag="x", name=f"x_{c0}")
            nc.gpsimd.dma_start(x_sb[:], xT[:, :, c0 : c0 + n])
            h_sb = hp.tile([P, kf, n], dt.bfloat16, tag="h", name=f"h_{c0}")

            # FFN1: hT[f, c] = relu(sum_d W1[d, f] * xT[d, c] + b1[f])
            for j in range(kf):
                if ci == 0:
                    # W1 j-slice loaded just ahead of first use; resident
                    # for all later chunks.
                    nc.sync.dma_start(w1_sb[:, j : j + 1, :], w1[:, j : j + 1, :])
                ps = pp.tile([P, n], dt.float32, tag="ps", name=f"ps1_{c0}_{j}")
                for k in range(kd):
                    nc.tensor.matmul(
                        ps[:],
                        w1_sb[:, j, k * P : (k + 1) * P],
                        x_sb[:, k, :],
                        start=(k == 0),
                        stop=(k == kd - 1),
                    )
                nc.scalar.activation(
                    h_sb[:, j : j + 1, :], ps[:], AF.Relu, bias=b1_sb[:, j : j + 1]
                )
                if ci == 0:
                    if j + 4 < kf:
                        nc.scalar.dma_start(
                            w2_sb[:, j + 4 : j + 5, :], w2[:, j + 4 : j + 5, :]
                        )
                    if j == min(16, kf - 1):
                        nc.scalar.dma_start(cw_sb[:], cw[:])

            # FFN2: yT[d, c] = (sum_f W2[f, d] * hT[f, c] + b2[d]) * cw[c]
            # j-outer / k-inner against the resident W2.
            for j in range(kd):
                ps = pp.tile([P, n], dt.float32, tag="ps", name=f"ps2_{c0}_{j}")
                for k in range(kf):
                    nc.tensor.matmul(
                        ps[:],
                        w2_sb[:, k, j * P : (j + 1) * P],
                        h_sb[:, k, :],
                        start=(k == 0),
                        stop=(k == kf - 1),
                    )
                y_sb = yp.tile([P, n], dt.float32, tag="y", name=f"y_{c0}_{j}")
                nc.vector.scalar_tensor_tensor(
                    y_sb[:],
                    ps[:],
                    b2_sb[:, j : j + 1],
                    cw_sb[:, c0 : c0 + n],
                    Alu.add,
                    Alu.mult,
                )
                nc.sync.dma_start(out[:, j, c0 : c0 + n], y_sb[:])
            c0 += n

    nc.finalize()
    return nc


def _get_program(C):
    if C not in _PROG_CACHE:
        _PROG_CACHE[C] = _build_program(C)
    return _PROG_CACHE[C]


def _maybe_install_ntff_shim():
    """If NTFF tracing is requested (BASS_TRACE=1) but the agent image lacks
    ``antenv.axon_hooks``, install an equivalent in-process shim driving the
    profiling C ABI of libaxon_pjrt.so directly (same implementation as
    trn_agent_boot). Without this, run_bass_kernel_spmd's trace branch raises
    ModuleNotFoundError. No-op when tracing is off or the real module exists.
    """
    if not os.environ.get("BASS_TRACE") or os.environ.get("BASS_NEVER_TRACE"):
        return
    try:
        import antenv.axon_hooks  # noqa: F401

        return
    except ImportError:
        pass
    import contextlib
    import ctypes
    import types

    so_path = "/opt/axon/libaxon_pjrt.so"
    if not os.path.exists(so_path):
        os.environ["BASS_NEVER_TRACE"] = "1"
        return
    lib = ctypes.CDLL(so_path)
    if not hasattr(lib, "axon_start_nrt_profile"):
        os.environ["BASS_NEVER_TRACE"] = "1"
        return
    lib.axon_start_nrt_profile.argtypes = [ctypes.POINTER(ctypes.c_int64), ctypes.c_size_t]
    lib.axon_start_nrt_profile.restype = ctypes.c_int64
    lib.axon_stop_nrt_profile.argtypes = [ctypes.c_char_p]
    lib.axon_stop_nrt_profile.restype = ctypes.c_int64

    @contextlib.contextmanager
    def _hook(output_dir, device_ids):
        import jax

        jax.devices()
        if device_ids:
            ids = (ctypes.c_int64 * len(device_ids))(*device_ids)
            rc = lib.axon_start_nrt_profile(ids, len(device_ids))
        else:
            rc = lib.axon_start_nrt_profile(None, 0)
        if rc != 0:
            raise RuntimeError(f"axon_start_nrt_profile rc={rc}")
        try:
            yield
        finally:
            lib.axon_stop_nrt_profile(str(output_dir).encode())

    mod = types.ModuleType("antenv.axon_hooks")
    mod.get_axon_ntff_profile_hook = lambda: _hook
    mod.set_axon_ntff_profile_hook = lambda h: None
    sys.modules["antenv.axon_hooks"] = mod

    import concourse.bass_utils as bu

    if getattr(bu.upload_artifacts, "__module__", "") == "concourse.bass_utils":
        bu.upload_artifacts = lambda tmpdir: f"local:{tmpdir}"


def kernel(**inputs):
    global LAST_RESULTS
    x = np.asarray(inputs["x"], np.float32)
    Wg = np.asarray(inputs["Wg"], np.float32)
    bg = np.asarray(inputs["bg"], np.float32)
    W1 = np.asarray(inputs["W1"], np.float32)
    b1 = np.asarray(inputs["b1"], np.float32)
    W2 = np.asarray(inputs["W2"], np.float32)
    b2 = np.asarray(inputs["b2"], np.float32)

    Bb, Ss, d = x.shape
    T = Bb * Ss
    xf = x.reshape(T, d)

    # --- gate + top-2 routing (host, fp32; this IS the sharding decision) ---
    logits = xf @ Wg + bg
    order = np.argsort(-logits, axis=1, kind="stable")[:, :TOPK]
    lsel = np.take_along_axis(logits, order, axis=1)
    wsel = np.exp(lsel - lsel[:, :1])
    wsel /= wsel.sum(axis=1, keepdims=True)

    idxs, cwts = [], []
    for e in range(E):
        m = order == e
        tok = np.nonzero(m.any(axis=1))[0]
        wt = np.where(m[tok, 0], wsel[tok, 0], wsel[tok, 1]).astype(np.float32)
        idxs.append(tok)
        cwts.append(wt)

    maxc = max(len(t) for t in idxs)
    C = max(P, -(-maxc // 8) * 8)  # capacity: 8-token alignment is enough
    nc = _get_program(C)

    # --- dispatch: build per-core (per-expert) shards ---
    bf16 = ml_dtypes.bfloat16
    xb = xf.astype(bf16)
    in_maps = []
    for e in range(E):
        tok = idxs[e]
        c = len(tok)
        xe = np.zeros((C, d), dtype=bf16)
        xe[:c] = xb[tok]
        xT = np.ascontiguousarray(xe.T.reshape(KD, P, C).transpose(1, 0, 2))
        w1m = np.ascontiguousarray(
            W1[e].astype(bf16).reshape(KD, P, KF, P).transpose(1, 2, 0, 3).reshape(P, KF, KD * P)
        )
        w2k = np.ascontiguousarray(W2[e].astype(bf16).reshape(KF, P, d).transpose(1, 0, 2))
        b1e = np.ascontiguousarray(b1[e].reshape(KF, P).T.astype(np.float32))
        b2e = np.ascontiguousarray(b2[e].reshape(KD, P).T.astype(np.float32))
        cwe = np.zeros((C,), np.float32)
        cwe[:c] = cwts[e]
        cwb = np.ascontiguousarray(np.broadcast_to(cwe[None, :], (P, C)))
        in_maps.append({"xT": xT, "w1": w1m, "b1": b1e, "w2": w2k, "b2": b2e, "cw": cwb})

    _maybe_install_ntff_shim()
    from concourse.bass_utils import run_bass_kernel_spmd

    res = run_bass_kernel_spmd(nc, in_maps, core_ids=list(range(E)))
    LAST_RESULTS = res

    # --- combine: scatter-add the weighted expert outputs ---
    outf = np.zeros((T, d), np.float32)
    for e in range(E):
        tok = idxs[e]
        c = len(tok)
        yT = np.asarray(res.results[e]["out"])  # [P, KD, C] f32
        ye = yT.transpose(2, 1, 0).reshape(C, d)
        outf[tok] += ye[:c]

    # --- aux loss (host, fp32) ---
    zmax = logits.max(axis=1, keepdims=True)
    pz = np.exp(logits - zmax)
    pz /= pz.sum(axis=1, keepdims=True)
    aux = np.float32((pz.mean(axis=0) ** 2).sum() * E)

    return outf.reshape(Bb, Ss, d), aux


# revision 26
# speedup vs baseline: 1.0010x; 1.0010x over previous
"""MoE layer (top-2 of 8 experts, d_model=1024, d_ff=4096) on 8 TRN2 NeuronCores.

Strategy: expert parallelism. The gate (x @ Wg + bg, top-2, softmax) is the
sharding decision and runs on host in fp32 (bit-stable vs the reference's
fp32 gate: min |logit2 - logit3| gap for these inputs is ~7e-5, far above
fp32 matmul rounding). Each core e receives the tokens routed to expert e
(padded to a common capacity C = max expert load, 8-aligned), W1/b1/W2/b2 of
expert e, and the per-token combine weight. The device computes

    yT = (relu(W1^T @ xT + b1)^T @ W2 + b2)^T * cw      (bf16 mm, fp32 acc)

entirely in transposed (feature-major) layout so both biases land on the
partition axis (scalar-engine activation bias) and the combine weight lands
on the free axis (one fused DVE op). Host scatter-adds the two expert
contributions per token (indices within one expert are unique, so fancy
indexing += is exact).
"""

import os
import sys

import numpy as np
import ml_dtypes

B, S, D, F, E, TOPK = 4, 2048, 1024, 4096, 8, 2
P = 128
KD = D // P  # 8  k-tiles over d_model
KF = F // P  # 32 k-tiles over d_ff

_PROG_CACHE: dict = {}
LAST_RESULTS = None  # BassKernelResults of the most recent device run


def _chunk_sizes(C):
    # Chunks of <=512 tokens (PSUM bank limit / max fp32 matmul free dim).
    # Weights are SBUF-resident, so a small tail chunk costs only its own
    # per-matmul overhead.
    sizes = []
    c = C
    while c > 0:
        n = min(512, c)
        sizes.append(n)
        c -= n
    return sizes


def _build_program(C, d=D, f=F):
    """One SPMD program: FFN for `C` tokens through one expert (transposed layout).

    DRAM inputs (per core):
      xT  [P, kd, C]    bf16   xT[p,k,c] = x[c, k*128+p]
      w1  [P, kf, kd*P] bf16   w1[p,j,k*128+q] = W1[k*128+p, j*128+q]  (m-major)
      b1  [P, kf]       f32    b1[p,j] = b1[j*128+p]
      w2  [P, kf, d]    bf16   w2[p,k,:] = W2[k*128+p, :]              (k-major)
      b2  [P, kd]       f32    b2[p,j] = b2[j*128+p]
      cw  [P, C]        f32    combine weight per token, replicated over partitions
    Output:
      out [P, kd, C]    f32    out[p,j,c] = y[c, j*128+p]
    """
    from contextlib import ExitStack

    import concourse.tile as tile
    from concourse import bacc, mybir

    kd = d // P
    kf = f // P
    dt = mybir.dt
    AF = mybir.ActivationFunctionType
    Alu = mybir.AluOpType

    nc = bacc.Bacc("TRN2", target_bir_lowering=False, debug=False, num_devices=E)
    xT = nc.dram_tensor("xT", [P, kd, C], dt.bfloat16, kind="ExternalInput").ap()
    w1 = nc.dram_tensor("w1", [P, kf, kd * P], dt.bfloat16, kind="ExternalInput").ap()
    b1 = nc.dram_tensor("b1", [P, kf], dt.float32, kind="ExternalInput").ap()
    w2 = nc.dram_tensor("w2", [P, kf, d], dt.bfloat16, kind="ExternalInput").ap()
    b2 = nc.dram_tensor("b2", [P, kd], dt.float32, kind="ExternalInput").ap()
    cw = nc.dram_tensor("cw", [P, C], dt.float32, kind="ExternalInput").ap()
    out = nc.dram_tensor("out", [P, kd, C], dt.float32, kind="ExternalOutput").ap()

    with tile.TileContext(nc) as tc, ExitStack() as ctx:
        xp = ctx.enter_context(tc.tile_pool(name="xp", bufs=2))
        hp = ctx.enter_context(tc.tile_pool(name="hp", bufs=1))
        yp = ctx.enter_context(tc.tile_pool(name="yp", bufs=3))
        cp = ctx.enter_context(tc.tile_pool(name="cp", bufs=1))
        pp = ctx.enter_context(tc.tile_pool(name="pp", bufs=8, space="PSUM"))

        # Both weight matrices are SBUF-resident (128KB/partition total).
        # Constants + W2 load on the scalar HWDGE ring, so the sync ring is
        # free for the latency-critical W1/x loads (first MM fires a few us
        # in; W2 is only needed ~70us in, once FFN2 starts). W2 is loaded in
        # 256KB slices — a single 8.4MB HWDGE DMA faults the device.
        b1_sb = cp.tile([P, kf], dt.float32)
        nc.scalar.dma_start(b1_sb[:], b1[:])
        b2_sb = cp.tile([P, kd], dt.float32)
        nc.scalar.dma_start(b2_sb[:], b2[:])
        cw_sb = cp.tile([P, C], dt.float32)
        nc.scalar.dma_start(cw_sb[:], cw[:])
        w1_sb = cp.tile([P, kf, kd * P], dt.bfloat16)
        w2_sb = cp.tile([P, kf, d], dt.bfloat16)
        # W2 loads are interleaved into chunk 0's eviction stream below
        # (slice j issued from the scalar queue after eviction j): they trail
        # W1 consumption instead of competing with the W1 stream for SDMA
        # bandwidth — chunk 0's W1 delivery is exactly at the SDMA capacity
        # edge, and any extra concurrent load there stalls FFN1 and
        # re-throttles the PE (measured both with an 8.4MB bulk load and
        # with a 4-slice lookahead).

        c0 = 0
        for ci, n in enumerate(_chunk_sizes(C)):
            # x on the gpsimd (SWDGE) ring: parallel with w1_0 on sync, so
            # the first matmul's two inputs don't serialize on one queue.
            x_sb = xp.tile([P, kd, n], dt.bfloat16, tag="x", name=f"x_{c0}")
            nc.gpsimd.dma_start(x_sb[:], xT[:, :, c0 : c0 + n])
            h_sb = hp.tile([P, kf, n], dt.bfloat16, tag="h", name=f"h_{c0}")

            # FFN1: hT[f, c] = relu(sum_d W1[d, f] * xT[d, c] + b1[f])
            for j in range(kf):
                if ci == 0:
                    # W1 j-slice loaded just ahead of first use; resident
                    # for all later chunks.
                    nc.sync.dma_start(w1_sb[:, j : j + 1, :], w1[:, j : j + 1, :])
                ps = pp.tile([P, n], dt.float32, tag="ps", name=f"ps1_{c0}_{j}")
                for k in range(kd):
                    nc.tensor.matmul(
                        ps[:],
                        w1_sb[:, j, k * P : (k + 1) * P],
                        x_sb[:, k, :],
                        start=(k == 0),
                        stop=(k == kd - 1),
                    )
                nc.scalar.activation(
                    h_sb[:, j : j + 1, :], ps[:], AF.Relu, bias=b1_sb[:, j : j + 1]
                )
                if ci == 0:
                    nc.scalar.dma_start(w2_sb[:, j : j + 1, :], w2[:, j : j + 1, :])

            # FFN2: yT[d, c] = (sum_f W2[f, d] * hT[f, c] + b2[d]) * cw[c]
            # j-outer / k-inner against the resident W2.
            for j in range(kd):
                ps = pp.tile([P, n], dt.float32, tag="ps", name=f"ps2_{c0}_{j}")
                for k in range(kf):
                    nc.tensor.matmul(
                        ps[:],
                        w2_sb[:, k, j * P : (j + 1) * P],
                        h_sb[:, k, :],
                        start=(k == 0),
                        stop=(k == kf - 1),
                    )
                y_sb = yp.tile([P, n], dt.float32, tag="y", name=f"y_{c0}_{j}")
                nc.vector.scalar_tensor_tensor(
                    y_sb[:],
                    ps[:],
                    b2_sb[:, j : j + 1],
                    cw_sb[:, c0 : c0 + n],
                    Alu.add,
                    Alu.mult,
                )
                nc.sync.dma_start(out[:, j, c0 : c0 + n], y_sb[:])
            c0 += n

    nc.finalize()
    return nc


def _get_program(C):
    if C not in _PROG_CACHE:
        _PROG_CACHE[C] = _build_program(C)
    return _PROG_CACHE[C]


def _maybe_install_ntff_shim():
    """If NTFF tracing is requested (BASS_TRACE=1) but the agent image lacks
    ``antenv.axon_hooks``, install an equivalent in-process shim driving the
    profiling C ABI of libaxon_pjrt.so directly (same implementation as
    trn_agent_boot). Without this, run_bass_kernel_spmd's trace branch raises
    ModuleNotFoundError. No-op when tracing is off or the real module exists.
    """
    if not os.environ.get("BASS_TRACE") or os.environ.get("BASS_NEVER_TRACE"):
        return
    try:
        import antenv.axon_hooks  # noqa: F401

        return
    except ImportError:
        pass
    import contextlib
    import ctypes
    import types

    so_path = "/opt/axon/libaxon_pjrt.so"
    if not os.path.exists(so_path):
        os.environ["BASS_NEVER_TRACE"] = "1"
        return
    lib = ctypes.CDLL(so_path)
    if not hasattr(lib, "axon_start_nrt_profile"):
        os.environ["BASS_NEVER_TRACE"] = "1"
        return
    lib.axon_start_nrt_profile.argtypes = [ctypes.POINTER(ctypes.c_int64), ctypes.c_size_t]
    lib.axon_start_nrt_profile.restype = ctypes.c_int64
    lib.axon_stop_nrt_profile.argtypes = [ctypes.c_char_p]
    lib.axon_stop_nrt_profile.restype = ctypes.c_int64

    @contextlib.contextmanager
    def _hook(output_dir, device_ids):
        import jax

        jax.devices()
        if device_ids:
            ids = (ctypes.c_int64 * len(device_ids))(*device_ids)
            rc = lib.axon_start_nrt_profile(ids, len(device_ids))
        else:
            rc = lib.axon_start_nrt_profile(None, 0)
        if rc != 0:
            raise RuntimeError(f"axon_start_nrt_profile rc={rc}")
        try:
            yield
        finally:
            lib.axon_stop_nrt_profile(str(output_dir).encode())

    mod = types.ModuleType("antenv.axon_hooks")
    mod.get_axon_ntff_profile_hook = lambda: _hook
    mod.set_axon_ntff_profile_hook = lambda h: None
    sys.modules["antenv.axon_hooks"] = mod

    import concourse.bass_utils as bu

    if getattr(bu.upload_artifacts, "__module__", "") == "concourse.bass_utils":
        bu.upload_artifacts = lambda tmpdir: f"local:{tmpdir}"


def kernel(**inputs):
    global LAST_RESULTS
    x = np.asarray(inputs["x"], np.float32)
    Wg = np.asarray(inputs["Wg"], np.float32)
    bg = np.asarray(inputs["bg"], np.float32)
    W1 = np.asarray(inputs["W1"], np.float32)
    b1 = np.asarray(inputs["b1"], np.float32)
    W2 = np.asarray(inputs["W2"], np.float32)
    b2 = np.asarray(inputs["b2"], np.float32)

    Bb, Ss, d = x.shape
    T = Bb * Ss
    xf = x.reshape(T, d)

    # --- gate + top-2 routing (host, fp32; this IS the sharding decision) ---
    logits = xf @ Wg + bg
    order = np.argsort(-logits, axis=1, kind="stable")[:, :TOPK]
    lsel = np.take_along_axis(logits, order, axis=1)
    wsel = np.exp(lsel - lsel[:, :1])
    wsel /= wsel.sum(axis=1, keepdims=True)

    idxs, cwts = [], []
    for e in range(E):
        m = order == e
        tok = np.nonzero(m.any(axis=1))[0]
        wt = np.where(m[tok, 0], wsel[tok, 0], wsel[tok, 1]).astype(np.float32)
        idxs.append(tok)
        cwts.append(wt)

    maxc = max(len(t) for t in idxs)
    C = max(P, -(-maxc // 8) * 8)  # capacity: 8-token alignment is enough
    nc = _get_program(C)

    # --- dispatch: build per-core (per-expert) shards ---
    bf16 = ml_dtypes.bfloat16
    xb = xf.astype(bf16)
    in_maps = []
    for e in range(E):
        tok = idxs[e]
        c = len(tok)
        xe = np.zeros((C, d), dtype=bf16)
        xe[:c] = xb[tok]
        xT = np.ascontiguousarray(xe.T.reshape(KD, P, C).transpose(1, 0, 2))
        w1m = np.ascontiguousarray(
            W1[e].astype(bf16).reshape(KD, P, KF, P).transpose(1, 2, 0, 3).reshape(P, KF, KD * P)
        )
        w2k = np.ascontiguousarray(W2[e].astype(bf16).reshape(KF, P, d).transpose(1, 0, 2))
        b1e = np.ascontiguousarray(b1[e].reshape(KF, P).T.astype(np.float32))
        b2e = np.ascontiguousarray(b2[e].reshape(KD, P).T.astype(np.float32))
        cwe = np.zeros((C,), np.float32)
        cwe[:c] = cwts[e]
        cwb = np.ascontiguousarray(np.broadcast_to(cwe[None, :], (P, C)))
        in_maps.append({"xT": xT, "w1": w1m, "b1": b1e, "w2": w2k, "b2": b2e, "cw": cwb})

    _maybe_install_ntff_shim()
    from concourse.bass_utils import run_bass_kernel_spmd

    res = run_bass_kernel_spmd(nc, in_maps, core_ids=list(range(E)))
    LAST_RESULTS = res

    # --- combine: scatter-add the weighted expert outputs ---
    outf = np.zeros((T, d), np.float32)
    for e in range(E):
        tok = idxs[e]
        c = len(tok)
        yT = np.asarray(res.results[e]["out"])  # [P, KD, C] f32
        ye = yT.transpose(2, 1, 0).reshape(C, d)
        outf[tok] += ye[:c]

    # --- aux loss (host, fp32) ---
    zmax = logits.max(axis=1, keepdims=True)
    pz = np.exp(logits - zmax)
    pz /= pz.sum(axis=1, keepdims=True)
    aux = np.float32((pz.mean(axis=0) ** 2).sum() * E)

    return outf.reshape(Bb, Ss, d), aux


# revision 27
# speedup vs baseline: 1.0051x; 1.0041x over previous
"""MoE layer (top-2 of 8 experts, d_model=1024, d_ff=4096) on 8 TRN2 NeuronCores.

Strategy: expert parallelism. The gate (x @ Wg + bg, top-2, softmax) is the
sharding decision and runs on host in fp32 (bit-stable vs the reference's
fp32 gate: min |logit2 - logit3| gap for these inputs is ~7e-5, far above
fp32 matmul rounding). Each core e receives the tokens routed to expert e
(padded to a common capacity C = max expert load, 8-aligned), W1/b1/W2/b2 of
expert e, and the per-token combine weight. The device computes

    yT = (relu(W1^T @ xT + b1)^T @ W2 + b2)^T * cw      (bf16 mm, fp32 acc)

entirely in transposed (feature-major) layout so both biases land on the
partition axis (scalar-engine activation bias) and the combine weight lands
on the free axis (one fused DVE op). Host scatter-adds the two expert
contributions per token (indices within one expert are unique, so fancy
indexing += is exact).
"""

import os
import sys

import numpy as np
import ml_dtypes

B, S, D, F, E, TOPK = 4, 2048, 1024, 4096, 8, 2
P = 128
KD = D // P  # 8  k-tiles over d_model
KF = F // P  # 32 k-tiles over d_ff

_PROG_CACHE: dict = {}
LAST_RESULTS = None  # BassKernelResults of the most recent device run


def _chunk_sizes(C):
    # Chunks of <=512 tokens (PSUM bank limit / max fp32 matmul free dim).
    # Weights are SBUF-resident, so a small tail chunk costs only its own
    # per-matmul overhead.
    sizes = []
    c = C
    while c > 0:
        n = min(512, c)
        sizes.append(n)
        c -= n
    return sizes


def _build_program(C, d=D, f=F):
    """One SPMD program: FFN for `C` tokens through one expert (transposed layout).

    DRAM inputs (per core):
      xT  [P, kd, C]    bf16   xT[p,k,c] = x[c, k*128+p]
      w1  [P, kf, kd*P] bf16   w1[p,j,k*128+q] = W1[k*128+p, j*128+q]  (m-major)
      b1  [P, kf]       f32    b1[p,j] = b1[j*128+p]
      w2  [P, kf, d]    bf16   w2[p,k,:] = W2[k*128+p, :]              (k-major)
      b2  [P, kd]       f32    b2[p,j] = b2[j*128+p]
      cw  [P, C]        f32    combine weight per token, replicated over partitions
    Output:
      out [P, kd, C]    f32    out[p,j,c] = y[c, j*128+p]
    """
    from contextlib import ExitStack

    import concourse.tile as tile
    from concourse import bacc, mybir

    kd = d // P
    kf = f // P
    dt = mybir.dt
    AF = mybir.ActivationFunctionType
    Alu = mybir.AluOpType

    nc = bacc.Bacc("TRN2", target_bir_lowering=False, debug=False, num_devices=E)
    xT = nc.dram_tensor("xT", [P, kd, C], dt.bfloat16, kind="ExternalInput").ap()
    w1 = nc.dram_tensor("w1", [P, kf, kd * P], dt.bfloat16, kind="ExternalInput").ap()
    b1 = nc.dram_tensor("b1", [P, kf], dt.float32, kind="ExternalInput").ap()
    w2 = nc.dram_tensor("w2", [P, kf, d], dt.bfloat16, kind="ExternalInput").ap()
    b2 = nc.dram_tensor("b2", [P, kd], dt.float32, kind="ExternalInput").ap()
    cw = nc.dram_tensor("cw", [P, C], dt.float32, kind="ExternalInput").ap()
    out = nc.dram_tensor("out", [P, kd, C], dt.float32, kind="ExternalOutput").ap()

    with tile.TileContext(nc) as tc, ExitStack() as ctx:
        xp = ctx.enter_context(tc.tile_pool(name="xp", bufs=2))
        hp = ctx.enter_context(tc.tile_pool(name="hp", bufs=1))
        yp = ctx.enter_context(tc.tile_pool(name="yp", bufs=3))
        cp = ctx.enter_context(tc.tile_pool(name="cp", bufs=1))
        pp = ctx.enter_context(tc.tile_pool(name="pp", bufs=8, space="PSUM"))

        # Both weight matrices are SBUF-resident (128KB/partition total).
        # Constants + W2 load on the scalar HWDGE ring, so the sync ring is
        # free for the latency-critical W1/x loads (first MM fires a few us
        # in; W2 is only needed ~70us in, once FFN2 starts). W2 is loaded in
        # 256KB slices — a single 8.4MB HWDGE DMA faults the device.
        b1_sb = cp.tile([P, kf], dt.float32)
        nc.scalar.dma_start(b1_sb[:], b1[:])
        b2_sb = cp.tile([P, kd], dt.float32)
        nc.scalar.dma_start(b2_sb[:], b2[:])
        cw_sb = cp.tile([P, C], dt.float32)
        nc.scalar.dma_start(cw_sb[:], cw[:])
        w1_sb = cp.tile([P, kf, kd * P], dt.bfloat16)
        w2_sb = cp.tile([P, kf, d], dt.bfloat16)
        # W2 loads are interleaved into chunk 0's eviction stream below:
        # issued from the scalar queue after eviction j, they trail W1
        # consumption instead of competing with the W1/x stream for SDMA
        # bandwidth (which starved FFN1 and re-throttled the PE).

        c0 = 0
        for ci, n in enumerate(_chunk_sizes(C)):
            x_sb = xp.tile([P, kd, n], dt.bfloat16, tag="x", name=f"x_{c0}")
            nc.sync.dma_start(x_sb[:], xT[:, :, c0 : c0 + n])
            h_sb = hp.tile([P, kf, n], dt.bfloat16, tag="h", name=f"h_{c0}")

            # FFN1: hT[f, c] = relu(sum_d W1[d, f] * xT[d, c] + b1[f])
            for j in range(kf):
                if ci == 0:
                    # W1 j-slice loaded just ahead of first use; resident
                    # for all later chunks.
                    nc.sync.dma_start(w1_sb[:, j : j + 1, :], w1[:, j : j + 1, :])
                ps = pp.tile([P, n], dt.float32, tag="ps", name=f"ps1_{c0}_{j}")
                for k in range(kd):
                    nc.tensor.matmul(
                        ps[:],
                        w1_sb[:, j, k * P : (k + 1) * P],
                        x_sb[:, k, :],
                        start=(k == 0),
                        stop=(k == kd - 1),
                    )
                nc.scalar.activation(
                    h_sb[:, j : j + 1, :], ps[:], AF.Relu, bias=b1_sb[:, j : j + 1]
                )
                if ci == 0:
                    nc.scalar.dma_start(w2_sb[:, j : j + 1, :], w2[:, j : j + 1, :])

            # FFN2: yT[d, c] = (sum_f W2[f, d] * hT[f, c] + b2[d]) * cw[c]
            # j-outer / k-inner against the resident W2.
            for j in range(kd):
                ps = pp.tile([P, n], dt.float32, tag="ps", name=f"ps2_{c0}_{j}")
                for k in range(kf):
                    nc.tensor.matmul(
                        ps[:],
                        w2_sb[:, k, j * P : (j + 1) * P],
                        h_sb[:, k, :],
                        start=(k == 0),
                        stop=(k == kf - 1),
                    )
                y_sb = yp.tile([P, n], dt.float32, tag="y", name=f"y_{c0}_{j}")
                nc.vector.scalar_tensor_tensor(
                    y_sb[:],
                    ps[:],
                    b2_sb[:, j : j + 1],
                    cw_sb[:, c0 : c0 + n],
                    Alu.add,
                    Alu.mult,
                )
                nc.sync.dma_start(out[:, j, c0 : c0 + n], y_sb[:])
            c0 += n

    nc.finalize()
    return nc


def _get_program(C):
    if C not in _PROG_CACHE:
        _PROG_CACHE[C] = _build_program(C)
    return _PROG_CACHE[C]


def _maybe_install_ntff_shim():
    """If NTFF tracing is requested (BASS_TRACE=1) but the agent image lacks
    ``antenv.axon_hooks``, install an equivalent in-process shim driving the
    profiling C ABI of libaxon_pjrt.so directly (same implementation as
    trn_agent_boot). Without this, run_bass_kernel_spmd's trace branch raises
    ModuleNotFoundError. No-op when tracing is off or the real module exists.
    """
    if not os.environ.get("BASS_TRACE") or os.environ.get("BASS_NEVER_TRACE"):
        return
    try:
        import antenv.axon_hooks  # noqa: F401

        return
    except ImportError:
        pass
    import contextlib
    import ctypes
    import types

    so_path = "/opt/axon/libaxon_pjrt.so"
    if not os.path.exists(so_path):
        os.environ["BASS_NEVER_TRACE"] = "1"
        return
    lib = ctypes.CDLL(so_path)
    if not hasattr(lib, "axon_start_nrt_profile"):
        os.environ["BASS_NEVER_TRACE"] = "1"
        return
    lib.axon_start_nrt_profile.argtypes = [ctypes.POINTER(ctypes.c_int64), ctypes.c_size_t]
    lib.axon_start_nrt_profile.restype = ctypes.c_int64
    lib.axon_stop_nrt_profile.argtypes = [ctypes.c_char_p]
    lib.axon_stop_nrt_profile.restype = ctypes.c_int64

    @contextlib.contextmanager
    def _hook(output_dir, device_ids):
        import jax

        jax.devices()
        if device_ids:
            ids = (ctypes.c_int64 * len(device_ids))(*device_ids)
            rc = lib.axon_start_nrt_profile(ids, len(device_ids))
        else:
            rc = lib.axon_start_nrt_profile(None, 0)
        if rc != 0:
            raise RuntimeError(f"axon_start_nrt_profile rc={rc}")
        try:
            yield
        finally:
            lib.axon_stop_nrt_profile(str(output_dir).encode())

    mod = types.ModuleType("antenv.axon_hooks")
    mod.get_axon_ntff_profile_hook = lambda: _hook
    mod.set_axon_ntff_profile_hook = lambda h: None
    sys.modules["antenv.axon_hooks"] = mod

    import concourse.bass_utils as bu

    if getattr(bu.upload_artifacts, "__module__", "") == "concourse.bass_utils":
        bu.upload_artifacts = lambda tmpdir: f"local:{tmpdir}"


def kernel(**inputs):
    global LAST_RESULTS
    x = np.asarray(inputs["x"], np.float32)
    Wg = np.asarray(inputs["Wg"], np.float32)
    bg = np.asarray(inputs["bg"], np.float32)
    W1 = np.asarray(inputs["W1"], np.float32)
    b1 = np.asarray(inputs["b1"], np.float32)
    W2 = np.asarray(inputs["W2"], np.float32)
    b2 = np.asarray(inputs["b2"], np.float32)

    Bb, Ss, d = x.shape
    T = Bb * Ss
    xf = x.reshape(T, d)

    # --- gate + top-2 routing (host, fp32; this IS the sharding decision) ---
    logits = xf @ Wg + bg
    order = np.argsort(-logits, axis=1, kind="stable")[:, :TOPK]
    lsel = np.take_along_axis(logits, order, axis=1)
    wsel = np.exp(lsel - lsel[:, :1])
    wsel /= wsel.sum(axis=1, keepdims=True)

    idxs, cwts = [], []
    for e in range(E):
        m = order == e
        tok = np.nonzero(m.any(axis=1))[0]
        wt = np.where(m[tok, 0], wsel[tok, 0], wsel[tok, 1]).astype(np.float32)
        idxs.append(tok)
        cwts.append(wt)

    maxc = max(len(t) for t in idxs)
    C = max(P, -(-maxc // 8) * 8)  # capacity: 8-token alignment is enough
    nc = _get_program(C)

    # --- dispatch: build per-core (per-expert) shards ---
    bf16 = ml_dtypes.bfloat16
    xb = xf.astype(bf16)
    in_maps = []
    for e in range(E):
        tok = idxs[e]
        c = len(tok)
        xe = np.zeros((C, d), dtype=bf16)
        xe[:c] = xb[tok]
        xT = np.ascontiguousarray(xe.T.reshape(KD, P, C).transpose(1, 0, 2))
        w1m = np.ascontiguousarray(
            W1[e].astype(bf16).reshape(KD, P, KF, P).transpose(1, 2, 0, 3).reshape(P, KF, KD * P)
        )
        w2k = np.ascontiguousarray(W2[e].astype(bf16).reshape(KF, P, d).transpose(1, 0, 2))
        b1e = np.ascontiguousarray(b1[e].reshape(KF, P).T.astype(np.float32))
        b2e = np.ascontiguousarray(b2[e].reshape(KD, P).T.astype(np.float32))
        cwe = np.zeros((C,), np.float32)
        cwe[:c] = cwts[e]
        cwb = np.ascontiguousarray(np.broadcast_to(cwe[None, :], (P, C)))
        in_maps.append({"xT": xT, "w1": w1m, "b1": b1e, "w2": w2k, "b2": b2e, "cw": cwb})

    _maybe_install_ntff_shim()
    from concourse.bass_utils import run_bass_kernel_spmd

    res = run_bass_kernel_spmd(nc, in_maps, core_ids=list(range(E)))
    LAST_RESULTS = res

    # --- combine: scatter-add the weighted expert outputs ---
    outf = np.zeros((T, d), np.float32)
    for e in range(E):
        tok = idxs[e]
        c = len(tok)
        yT = np.asarray(res.results[e]["out"])  # [P, KD, C] f32
        ye = yT.transpose(2, 1, 0).reshape(C, d)
        outf[tok] += ye[:c]

    # --- aux loss (host, fp32) ---
    zmax = logits.max(axis=1, keepdims=True)
    pz = np.exp(logits - zmax)
    pz /= pz.sum(axis=1, keepdims=True)
    aux = np.float32((pz.mean(axis=0) ** 2).sum() * E)

    return outf.reshape(Bb, Ss, d), aux
